# revision 12
# baseline (speedup 1.0000x reference)
"""Trainium2 Bass kernel for a 2-layer GCN (PyG GCNConv semantics).

Strategy (8 NeuronCores, node-sharded / graph parallel):
  - Core c owns nodes [c*12500, (c+1)*12500), padded to 12544 = 98 windows x 128.
  - Using linearity of GCNConv, both layers become:
        z = dinv * (segment_sum_{e: dst=d} table[src_e] + table[d])  (+bias)
    with  table_1 = dinv * (x @ W1)         (64 wide)
          h       = tanh(z_1)
          table_2 = (dinv * h) @ W2         (2 wide)
          h2      = tanh(z_2)
          out     = h2 @ Wc + bc
    so there is NO per-edge scalar work at all - just gather + segmented sum.
  - Each core computes the table rows for its own nodes, AllGathers the full
    table to DRAM, then processes its in-edges (sorted by dst) in 128-node
    windows: per 128 edges one [128,1]-offset indirect DMA gathers the source
    rows (the only reliable data-dependent DMA on this platform), and a
    0/1-matrix matmul accumulates the segment sums for the window's 128
    nodes in PSUM.
  - The graph is known at program-build time, so every window emits exactly
    ceil(max-over-cores edge count / 128) tiles; padding lanes carry
    slot id 512 so their one-hot column is all-zero (contribute nothing).

Host-side work is restricted to graph partitioning / index packing (sorting
edges by destination, window slot assignment, degree counts) and final
output reassembly.
"""

import sys

sys.path.insert(0, "/opt/trn_rl_repo")

import numpy as np
from contextlib import ExitStack

from concourse import bacc, mybir, tile
from concourse import bass_utils
from concourse.bass import IndirectOffsetOnAxis
from concourse.masks import make_identity

F32 = np.float32

# Problem geometry (hardcoded per contest contract).
N = 100000
F_IN = 128
HID = 64
OUT2 = 2
NCLS = 4
NCORES = 8
NLOC = N // NCORES          # 12500 nodes per core
WINS = (NLOC + 127) // 128  # 98 windows of 128 nodes
NPAD = WINS * 128           # 12544 rows per core in the padded table
SB = 12                     # one-hot build batch (tiles per DVE op)
PAD_REL = 512.0             # slot id for padding edges -> all-zero one-hot col

_prog_cache: dict = {}


def _build_program(tiles_w, with_b1: bool, with_b2: bool, single_core: bool = False):
    """Build + compile the SPMD Bass program.

    tiles_w: per-window 128-edge tile counts (len WINS tuple, shared by all
    cores = max over cores).  single_core=True builds a collective-free
    variant (AllGather modeled as 8 local DRAM copies) for TimelineSim.
    """
    tiles_w = tuple(int(t) for t in tiles_w)
    ntiles = sum(tiles_w)
    tstart = np.concatenate([[0], np.cumsum(tiles_w)]).astype(int)

    dt = mybir.dt
    nc = bacc.Bacc(
        "TRN2",
        target_bir_lowering=False,
        debug=False,
        enable_asserts=False,
        num_devices=1 if single_core else NCORES,
    )

    # ------------- I/O -------------
    xT = nc.dram_tensor("xT", [128, NPAD], dt.float32, kind="ExternalInput")
    offs = nc.dram_tensor("offs", [128, ntiles], dt.int32, kind="ExternalInput")
    rel = nc.dram_tensor("rel", [128, ntiles], dt.float32, kind="ExternalInput")
    deg = nc.dram_tensor("deg", [128, WINS], dt.float32, kind="ExternalInput")
    W1t = nc.dram_tensor("W1t", [128, HID], dt.float32, kind="ExternalInput")
    W2t = nc.dram_tensor("W2t", [HID, OUT2], dt.float32, kind="ExternalInput")
    Wct = nc.dram_tensor("Wct", [OUT2, NCLS], dt.float32, kind="ExternalInput")
    bct = nc.dram_tensor("bct", [NCLS, 1], dt.float32, kind="ExternalInput")
    if with_b1:
        b1t = nc.dram_tensor("b1t", [128, HID], dt.float32, kind="ExternalInput")
    if with_b2:
        b2t = nc.dram_tensor("b2t", [128, OUT2], dt.float32, kind="ExternalInput")

    outT = nc.dram_tensor("outT", [NCLS, NPAD], dt.float32, kind="ExternalOutput")
    h2o = nc.dram_tensor("h2o", [128, WINS * OUT2], dt.float32, kind="ExternalOutput")

    # Internal tables. Shard layout: row (p*WINS + w) holds node w*128+p.
    u_shard = nc.dram_tensor("u_shard", [NPAD, HID], dt.float32, kind="Internal")
    u_tab = nc.dram_tensor(
        "u_tab", [NCORES * NPAD, HID], dt.float32, kind="Internal",
        addr_space="Shared",
    )
    t2_shard = nc.dram_tensor("t2_shard", [NPAD, OUT2], dt.float32, kind="Internal")
    t2_tab = nc.dram_tensor(
        "t2_tab", [NCORES * NPAD, OUT2], dt.float32, kind="Internal",
        addr_space="Shared",
    )

    groups = [list(range(NCORES))]

    with tile.TileContext(nc) as tc, ExitStack() as ctx:
        const = ctx.enter_context(tc.tile_pool(name="const", bufs=1))

        # --- constants / persistent SBUF state ---
        W1_s = const.tile([128, HID], dt.float32)
        nc.sync.dma_start(W1_s[:], W1t.ap())
        W2_s = const.tile([HID, OUT2], dt.float32)
        nc.sync.dma_start(W2_s[:], W2t.ap())
        Wc_s = const.tile([OUT2, NCLS], dt.float32)
        nc.sync.dma_start(Wc_s[:], Wct.ap())
        bc_s = const.tile([NCLS, 1], dt.float32)
        nc.sync.dma_start(bc_s[:], bct.ap())
        if with_b1:
            b1_s = const.tile([128, HID], dt.float32)
            nc.sync.dma_start(b1_s[:], b1t.ap())
        if with_b2:
            b2_s = const.tile([128, OUT2], dt.float32)
            nc.sync.dma_start(b2_s[:], b2t.ap())

        offs_s = const.tile([128, ntiles], dt.int32)
        nc.sync.dma_start(offs_s[:], offs.ap())
        rel_s = const.tile([128, ntiles], dt.float32)
        nc.sync.dma_start(rel_s[:], rel.ap())

        deg_s = const.tile([128, WINS], dt.float32)
        nc.sync.dma_start(deg_s[:], deg.ap())
        sqd = const.tile([128, WINS], dt.float32)
        nc.scalar.activation(sqd[:], deg_s[:], mybir.ActivationFunctionType.Sqrt)
        dinv = const.tile([128, WINS], dt.float32)
        nc.vector.reciprocal(dinv[:], sqd[:])

        # iota pattern repeated SB times: iota_rep[p, b*128 + s] = s
        iota16 = const.tile([128, SB * 128], dt.int16)
        nc.gpsimd.iota(iota16[:], pattern=[[0, SB], [1, 128]], base=0,
                       channel_multiplier=0)
        iota_f = const.tile([128, SB * 128], dt.float32)
        nc.vector.tensor_copy(iota_f[:], iota16[:])

        ident = const.tile([128, 128], dt.float32)
        make_identity(nc, ident[:])

        u_acc = const.tile([128, WINS * HID], dt.float32)     # own table rows (L1)
        t2_acc = const.tile([128, WINS * OUT2], dt.float32)   # own table rows (L2)
        h2_acc = const.tile([128, WINS * OUT2], dt.float32)   # layer-2 tanh output
        out_acc = const.tile([NCLS, NPAD], dt.float32)        # final logits (T)

        # ---------------- Phase A: table_1 = dinv * (x @ W1) ----------------
        with tc.tile_pool(name="phA_ps", bufs=4, space="PSUM") as psA, \
             tc.tile_pool(name="phA_sb", bufs=1) as sbA:
            xT_s = sbA.tile([128, NPAD], dt.float32)
            nc.sync.dma_start(xT_s[:], xT.ap())
            for w in range(WINS):
                ps = psA.tile([128, HID], dt.float32)
                nc.tensor.matmul(
                    ps[:], lhsT=xT_s[:, w * 128:(w + 1) * 128], rhs=W1_s[:],
                    start=True, stop=True,
                )
                nc.vector.tensor_scalar_mul(
                    u_acc[:, w * HID:(w + 1) * HID], ps[:], dinv[:, w:w + 1]
                )
        # publish + allgather the layer-1 table
        nc.gpsimd.dma_start(u_shard.ap(), u_acc[:])
        if single_core:
            for c in range(NCORES):
                nc.sync.dma_start(
                    u_tab.ap()[c * NPAD:(c + 1) * NPAD, :], u_shard.ap()
                )
        else:
            nc.gpsimd.collective_compute(
                "AllGather", mybir.AluOpType.bypass, replica_groups=groups,
                ins=[u_shard.ap()], outs=[u_tab.ap()],
            )

        # ---------------- Aggregation helper ----------------
        def aggregate(tab_ap, width, epilogue):
            with tc.tile_pool(name="agg_g", bufs=5) as gpool, \
                 tc.tile_pool(name="agg_s", bufs=4) as spool, \
                 tc.tile_pool(name="agg_ps", bufs=4, space="PSUM") as pspool, \
                 tc.tile_pool(name="agg_eps", bufs=2, space="PSUM") as pseps, \
                 tc.tile_pool(name="agg_ep", bufs=3) as eppool:
                for w in range(WINS):
                    tw = tiles_w[w]
                    base = tstart[w]
                    mg = gpool.tile([128, tw * width], dt.float32, tag="mg")
                    for t in range(tw):
                        nc.gpsimd.indirect_dma_start(
                            out=mg[:, t * width:(t + 1) * width],
                            out_offset=None,
                            in_=tab_ap,
                            in_offset=IndirectOffsetOnAxis(
                                ap=offs_s[:, base + t:base + t + 1], axis=0
                            ),
                        )
                    ps = pspool.tile([128, width], dt.float32, tag="acc")
                    t = 0
                    while t < tw:
                        nb = min(SB, tw - t)
                        s_oh = spool.tile([128, SB * 128], dt.float32, tag="oh")
                        nc.vector.tensor_tensor(
                            out=s_oh[:, :nb * 128],
                            in0=iota_f[:, :nb * 128],
                            in1=rel_s[:, base + t:base + t + nb]
                                .to_broadcast([128, nb, 128]),
                            op=mybir.AluOpType.is_equal,
                        )
                        for k in range(nb):
                            nc.tensor.matmul(
                                ps[:],
                                lhsT=s_oh[:, k * 128:(k + 1) * 128],
                                rhs=mg[:, (t + k) * width:(t + k + 1) * width],
                                start=(t + k == 0),
                                stop=(t + k == tw - 1),
                            )
                        t += nb
                    epilogue(w, ps, pseps, eppool)

        # ---------------- Layer 1 ----------------
        def epi1(w, ps, pseps, eppool):
            z = eppool.tile([128, HID], dt.float32, tag="z1")
            nc.vector.tensor_add(z[:], ps[:], u_acc[:, w * HID:(w + 1) * HID])
            h = eppool.tile([128, HID], dt.float32, tag="h1")
            if with_b1:
                zz = eppool.tile([128, HID], dt.float32, tag="zz1")
                nc.vector.tensor_scalar_mul(zz[:], z[:], dinv[:, w:w + 1])
                nc.vector.tensor_add(zz[:], zz[:], b1_s[:])
                nc.scalar.activation(h[:], zz[:], mybir.ActivationFunctionType.Tanh)
            else:
                nc.scalar.activation(
                    h[:], z[:], mybir.ActivationFunctionType.Tanh,
                    scale=dinv[:, w:w + 1],
                )
            # table_2 row block: (dinv * h) @ W2, via PE transpose
            th = eppool.tile([128, HID], dt.float32, tag="th")
            nc.vector.tensor_scalar_mul(th[:], h[:], dinv[:, w:w + 1])
            pt = pseps.tile([HID, 128], dt.float32, tag="tr1")
            nc.tensor.transpose(pt[:], th[:], ident[:])
            thT = eppool.tile([HID, 128], dt.float32, tag="thT")
            nc.vector.tensor_copy(thT[:], pt[:])
            p2 = pseps.tile([128, OUT2], dt.float32, tag="t2")
            nc.tensor.matmul(p2[:], lhsT=thT[:], rhs=W2_s[:], start=True, stop=True)
            nc.vector.tensor_copy(t2_acc[:, w * OUT2:(w + 1) * OUT2], p2[:])

        aggregate(u_tab.ap(), HID, epi1)

        # publish + allgather the layer-2 table
        nc.gpsimd.dma_start(t2_shard.ap(), t2_acc[:])
        if single_core:
            for c in range(NCORES):
                nc.sync.dma_start(
                    t2_tab.ap()[c * NPAD:(c + 1) * NPAD, :], t2_shard.ap()
                )
        else:
            nc.gpsimd.collective_compute(
                "AllGather", mybir.AluOpType.bypass, replica_groups=groups,
                ins=[t2_shard.ap()], outs=[t2_tab.ap()],
            )

        # ---------------- Layer 2 ----------------
        def epi2(w, ps, pseps, eppool):
            z = eppool.tile([128, OUT2], dt.float32, tag="z2")
            nc.vector.tensor_add(z[:], ps[:], t2_acc[:, w * OUT2:(w + 1) * OUT2])
            h2 = h2_acc[:, w * OUT2:(w + 1) * OUT2]
            if with_b2:
                zz = eppool.tile([128, OUT2], dt.float32, tag="zz2")
                nc.vector.tensor_scalar_mul(zz[:], z[:], dinv[:, w:w + 1])
                nc.vector.tensor_add(zz[:], zz[:], b2_s[:])
                nc.scalar.activation(h2, zz[:], mybir.ActivationFunctionType.Tanh)
            else:
                nc.scalar.activation(
                    h2, z[:], mybir.ActivationFunctionType.Tanh,
                    scale=dinv[:, w:w + 1],
                )
            # out = h2 @ Wc + bc, via PE transpose
            pt = pseps.tile([OUT2, 128], dt.float32, tag="tr2")
            nc.tensor.transpose(pt[:], h2, ident[:])
            h2T = eppool.tile([OUT2, 128], dt.float32, tag="h2T")
            nc.vector.tensor_copy(h2T[:], pt[:])
            pf = pseps.tile([NCLS, 128], dt.float32, tag="fin")
            nc.tensor.matmul(pf[:], lhsT=Wc_s[:], rhs=h2T[:], start=True, stop=True)
            nc.scalar.activation(
                out_acc[:, w * 128:(w + 1) * 128], pf[:],
                mybir.ActivationFunctionType.Identity, bias=bc_s[:],
            )

        aggregate(t2_tab.ap(), OUT2, epi2)

        # ---------------- outputs ----------------
        nc.sync.dma_start(outT.ap(), out_acc[:])
        nc.sync.dma_start(h2o.ap(), h2_acc[:])

    nc.compile()
    return nc


def _get_program(tiles_w, with_b1: bool, with_b2: bool):
    key = (tuple(tiles_w), with_b1, with_b2)
    if key not in _prog_cache:
        _prog_cache[key] = _build_program(tiles_w, with_b1, with_b2)
    return _prog_cache[key]


def _window_counts(dst):
    owner = dst // NLOC
    local = dst - owner * NLOC
    gw = owner * WINS + (local >> 7)
    cnts = np.bincount(gw, minlength=NCORES * WINS).reshape(NCORES, WINS)
    return cnts


def _preprocess(x, edge_index, deg_full, tiles_w):
    """Per-core inputs: xT, offs, rel, deg layouts. Pure index/layout work."""
    tiles_w = np.asarray(tiles_w, np.int64)
    ntiles = int(tiles_w.sum())
    tstart = np.zeros(WINS, np.int64)
    np.cumsum(tiles_w[:-1], out=tstart[1:])

    src = np.asarray(edge_index[0], dtype=np.int64)
    dst = np.asarray(edge_index[1], dtype=np.int64)
    order = np.argsort(dst, kind="stable")
    ds = dst[order]
    ss = src[order]
    core_starts = np.searchsorted(ds, np.arange(NCORES + 1) * NLOC)

    per_core = []
    for c in range(NCORES):
        lo, hi = int(core_starts[c]), int(core_starts[c + 1])
        ldst = ds[lo:hi] - c * NLOC
        lsrc = ss[lo:hi]
        w = ldst >> 7
        relv = (ldst & 127).astype(np.float32)
        cnt = np.bincount(w, minlength=WINS)
        if np.any(cnt > tiles_w * 128):
            raise ValueError("window overflow vs tiles_w")
        wstart = np.zeros(WINS, np.int64)
        np.cumsum(cnt[:-1], out=wstart[1:])
        pos = np.arange(hi - lo, dtype=np.int64) - wstart[w]

        # edge j of window w -> tile t=j//128, partition p=j%128, at column
        # tstart[w] + t of the [128, ntiles] SBUF layout
        col = tstart[w] + (pos >> 7)
        row_p = pos & 127

        oc = lsrc // NLOC
        ol = lsrc % NLOC
        trow = (oc * NPAD + (ol & 127) * WINS + (ol >> 7)).astype(np.int32)

        offs_sb = np.zeros((128, ntiles), np.int32)
        rel_sb = np.full((128, ntiles), PAD_REL, np.float32)
        offs_sb[row_p, col] = trow
        rel_sb[row_p, col] = relv

        deg_lay = np.ones((128, WINS), np.float32)
        l = np.arange(NLOC)
        deg_lay[l & 127, l >> 7] = deg_full[c * NLOC:(c + 1) * NLOC]

        xTc = np.zeros((128, NPAD), np.float32)
        xTc[:, :NLOC] = np.asarray(x[c * NLOC:(c + 1) * NLOC], np.float32).T

        per_core.append(dict(xT=xTc, offs=offs_sb, rel=rel_sb, deg=deg_lay))
    return per_core


def kernel(x, edge_index, W1, b1, W2, b2, Wc, bc, **_unused):
    x = np.asarray(x, np.float32)
    W1 = np.asarray(W1, np.float32)
    b1 = np.asarray(b1, np.float32)
    W2 = np.asarray(W2, np.float32)
    b2 = np.asarray(b2, np.float32)
    Wc = np.asarray(Wc, np.float32)
    bc = np.asarray(bc, np.float32)
    dst = np.asarray(edge_index[1], dtype=np.int64)

    deg_full = (np.bincount(dst, minlength=N) + 1).astype(np.float32)

    cnts = _window_counts(dst)
    tiles_w = tuple(
        max(1, int(t)) for t in np.ceil(cnts.max(axis=0) / 128).astype(int)
    )

    with_b1 = bool(np.any(b1))
    with_b2 = bool(np.any(b2))
    nc = _get_program(tiles_w, with_b1, with_b2)

    per_core = _preprocess(x, edge_index, deg_full, tiles_w)

    in_maps = []
    for c in range(NCORES):
        m = dict(
            xT=per_core[c]["xT"],
            offs=per_core[c]["offs"],
            rel=per_core[c]["rel"],
            deg=per_core[c]["deg"],
            W1t=W1,
            W2t=W2,
            Wct=Wc,
            bct=bc.reshape(NCLS, 1),
        )
        if with_b1:
            m["b1t"] = np.broadcast_to(b1, (128, HID)).copy()
        if with_b2:
            m["b2t"] = np.broadcast_to(b2, (128, OUT2)).copy()
        in_maps.append(m)

    res = bass_utils.run_bass_kernel_spmd(nc, in_maps, core_ids=list(range(NCORES)))

    out = np.empty((N, NCLS), np.float32)
    h2 = np.empty((N, OUT2), np.float32)
    for c in range(NCORES):
        r = res.results[c]
        out[c * NLOC:(c + 1) * NLOC] = r["outT"].T[:NLOC]
        h2c = r["h2o"].reshape(128, WINS, OUT2).transpose(1, 0, 2).reshape(NPAD, OUT2)
        h2[c * NLOC:(c + 1) * NLOC] = h2c[:NLOC]
    return (out, h2)


# revision 13
# speedup vs baseline: 1.0001x; 1.0001x over previous
"""Trainium2 Bass kernel for a 2-layer GCN (PyG GCNConv semantics).

Strategy (8 NeuronCores, node-sharded / graph parallel):
  - Core c owns nodes [c*12500, (c+1)*12500), padded to 12544 = 98 windows x 128.
  - Using linearity of GCNConv, both layers become:
        z = dinv * (segment_sum_{e: dst=d} table[src_e] + table[d])  (+bias)
    with  table_1 = dinv * (x @ W1)         (64 wide)
          h       = tanh(z_1)
          table_2 = (dinv * h) @ W2         (2 wide)
          h2      = tanh(z_2)
          out     = h2 @ Wc + bc
    so there is NO per-edge scalar work at all - just gather + segmented sum.
  - Each core computes the table rows for its own nodes, AllGathers the full
    table to DRAM, then processes its in-edges (sorted by dst) in 128-node
    windows: per 128 edges one [128,1]-offset indirect DMA gathers the source
    rows (the only reliable data-dependent DMA on this platform), and a
    0/1-matrix matmul accumulates the segment sums for the window's 128
    nodes in PSUM.
  - The graph is known at program-build time, so every window emits exactly
    ceil(max-over-cores edge count / 128) tiles; padding lanes carry
    slot id 512 so their one-hot column is all-zero (contribute nothing).

Host-side work is restricted to graph partitioning / index packing (sorting
edges by destination, window slot assignment, degree counts) and final
output reassembly.
"""

import sys

sys.path.insert(0, "/opt/trn_rl_repo")

import numpy as np
from contextlib import ExitStack

from concourse import bacc, mybir, tile
from concourse import bass_utils
from concourse.bass import IndirectOffsetOnAxis
from concourse.masks import make_identity

F32 = np.float32

# Problem geometry (hardcoded per contest contract).
N = 100000
F_IN = 128
HID = 64
OUT2 = 2
NCLS = 4
NCORES = 8
NLOC = N // NCORES          # 12500 nodes per core
WINS = (NLOC + 127) // 128  # 98 windows of 128 nodes
NPAD = WINS * 128           # 12544 rows per core in the padded table
SB = 12                     # one-hot build batch (tiles per DVE op)
PAD_REL = 512.0             # slot id for padding edges -> all-zero one-hot col

_prog_cache: dict = {}


def _build_program(tiles_w, with_b1: bool, with_b2: bool, single_core: bool = False):
    """Build + compile the SPMD Bass program.

    tiles_w: per-window 128-edge tile counts (len WINS tuple, shared by all
    cores = max over cores).  single_core=True builds a collective-free
    variant (AllGather modeled as 8 local DRAM copies) for TimelineSim.
    """
    tiles_w = tuple(int(t) for t in tiles_w)
    ntiles = sum(tiles_w)
    tstart = np.concatenate([[0], np.cumsum(tiles_w)]).astype(int)

    dt = mybir.dt
    nc = bacc.Bacc(
        "TRN2",
        target_bir_lowering=False,
        debug=False,
        enable_asserts=False,
        num_devices=1 if single_core else NCORES,
    )

    # ------------- I/O -------------
    xT = nc.dram_tensor("xT", [128, NPAD], dt.float32, kind="ExternalInput")
    offs = nc.dram_tensor("offs", [128, ntiles], dt.int32, kind="ExternalInput")
    rel = nc.dram_tensor("rel", [128, ntiles], dt.float32, kind="ExternalInput")
    deg = nc.dram_tensor("deg", [128, WINS], dt.float32, kind="ExternalInput")
    W1t = nc.dram_tensor("W1t", [128, HID], dt.float32, kind="ExternalInput")
    W2t = nc.dram_tensor("W2t", [HID, OUT2], dt.float32, kind="ExternalInput")
    Wct = nc.dram_tensor("Wct", [OUT2, NCLS], dt.float32, kind="ExternalInput")
    bct = nc.dram_tensor("bct", [NCLS, 1], dt.float32, kind="ExternalInput")
    if with_b1:
        b1t = nc.dram_tensor("b1t", [128, HID], dt.float32, kind="ExternalInput")
    if with_b2:
        b2t = nc.dram_tensor("b2t", [128, OUT2], dt.float32, kind="ExternalInput")

    outT = nc.dram_tensor("outT", [NCLS, NPAD], dt.float32, kind="ExternalOutput")
    h2o = nc.dram_tensor("h2o", [128, WINS * OUT2], dt.float32, kind="ExternalOutput")

    # Internal tables. Shard layout: row (p*WINS + w) holds node w*128+p.
    u_shard = nc.dram_tensor("u_shard", [NPAD, HID], dt.float32, kind="Internal")
    u_tab = nc.dram_tensor(
        "u_tab", [NCORES * NPAD, HID], dt.float32, kind="Internal",
        addr_space="Shared",
    )
    t2_shard = nc.dram_tensor("t2_shard", [NPAD, OUT2], dt.float32, kind="Internal")
    t2_tab = nc.dram_tensor(
        "t2_tab", [NCORES * NPAD, OUT2], dt.float32, kind="Internal",
        addr_space="Shared",
    )

    groups = [list(range(NCORES))]

    with tile.TileContext(nc) as tc, ExitStack() as ctx:
        const = ctx.enter_context(tc.tile_pool(name="const", bufs=1))

        # --- constants / persistent SBUF state ---
        W1_s = const.tile([128, HID], dt.float32)
        nc.sync.dma_start(W1_s[:], W1t.ap())
        W2_s = const.tile([HID, OUT2], dt.float32)
        nc.sync.dma_start(W2_s[:], W2t.ap())
        Wc_s = const.tile([OUT2, NCLS], dt.float32)
        nc.sync.dma_start(Wc_s[:], Wct.ap())
        bc_s = const.tile([NCLS, 1], dt.float32)
        nc.sync.dma_start(bc_s[:], bct.ap())
        if with_b1:
            b1_s = const.tile([128, HID], dt.float32)
            nc.sync.dma_start(b1_s[:], b1t.ap())
        if with_b2:
            b2_s = const.tile([128, OUT2], dt.float32)
            nc.sync.dma_start(b2_s[:], b2t.ap())

        offs_s = const.tile([128, ntiles], dt.int32)
        nc.sync.dma_start(offs_s[:], offs.ap())
        rel_s = const.tile([128, ntiles], dt.float32)
        nc.sync.dma_start(rel_s[:], rel.ap())

        deg_s = const.tile([128, WINS], dt.float32)
        nc.sync.dma_start(deg_s[:], deg.ap())
        sqd = const.tile([128, WINS], dt.float32)
        nc.scalar.activation(sqd[:], deg_s[:], mybir.ActivationFunctionType.Sqrt)
        dinv = const.tile([128, WINS], dt.float32)
        nc.vector.reciprocal(dinv[:], sqd[:])

        # iota pattern repeated SB times: iota_rep[p, b*128 + s] = s
        iota16 = const.tile([128, SB * 128], dt.int16)
        nc.gpsimd.iota(iota16[:], pattern=[[0, SB], [1, 128]], base=0,
                       channel_multiplier=0)
        iota_f = const.tile([128, SB * 128], dt.float32)
        nc.vector.tensor_copy(iota_f[:], iota16[:])

        ident = const.tile([128, 128], dt.float32)
        make_identity(nc, ident[:])

        u_acc = const.tile([128, WINS * HID], dt.float32)     # own table rows (L1)
        t2_acc = const.tile([128, WINS * OUT2], dt.float32)   # own table rows (L2)
        h2_acc = const.tile([128, WINS * OUT2], dt.float32)   # layer-2 tanh output
        out_acc = const.tile([NCLS, NPAD], dt.float32)        # final logits (T)

        # ---------------- Phase A: table_1 = dinv * (x @ W1) ----------------
        with tc.tile_pool(name="phA_ps", bufs=4, space="PSUM") as psA, \
             tc.tile_pool(name="phA_sb", bufs=1) as sbA:
            xT_s = sbA.tile([128, NPAD], dt.float32)
            nc.sync.dma_start(xT_s[:], xT.ap())
            for w in range(WINS):
                ps = psA.tile([128, HID], dt.float32)
                nc.tensor.matmul(
                    ps[:], lhsT=xT_s[:, w * 128:(w + 1) * 128], rhs=W1_s[:],
                    start=True, stop=True,
                )
                nc.vector.tensor_scalar_mul(
                    u_acc[:, w * HID:(w + 1) * HID], ps[:], dinv[:, w:w + 1]
                )
        # publish + allgather the layer-1 table (HWDGE; no cast needed)
        nc.sync.dma_start(u_shard.ap(), u_acc[:])
        if single_core:
            for c in range(NCORES):
                nc.sync.dma_start(
                    u_tab.ap()[c * NPAD:(c + 1) * NPAD, :], u_shard.ap()
                )
        else:
            nc.gpsimd.collective_compute(
                "AllGather", mybir.AluOpType.bypass, replica_groups=groups,
                ins=[u_shard.ap()], outs=[u_tab.ap()],
            )

        # ---------------- Aggregation helper ----------------
        def aggregate(tab_ap, width, epilogue):
            with tc.tile_pool(name="agg_g", bufs=5) as gpool, \
                 tc.tile_pool(name="agg_s", bufs=4) as spool, \
                 tc.tile_pool(name="agg_ps", bufs=4, space="PSUM") as pspool, \
                 tc.tile_pool(name="agg_eps", bufs=2, space="PSUM") as pseps, \
                 tc.tile_pool(name="agg_ep", bufs=3) as eppool:
                for w in range(WINS):
                    tw = tiles_w[w]
                    base = tstart[w]
                    mg = gpool.tile([128, tw * width], dt.float32, tag="mg")
                    for t in range(tw):
                        nc.gpsimd.indirect_dma_start(
                            out=mg[:, t * width:(t + 1) * width],
                            out_offset=None,
                            in_=tab_ap,
                            in_offset=IndirectOffsetOnAxis(
                                ap=offs_s[:, base + t:base + t + 1], axis=0
                            ),
                        )
                    ps = pspool.tile([128, width], dt.float32, tag="acc")
                    t = 0
                    while t < tw:
                        nb = min(SB, tw - t)
                        s_oh = spool.tile([128, SB * 128], dt.float32, tag="oh")
                        nc.vector.tensor_tensor(
                            out=s_oh[:, :nb * 128],
                            in0=iota_f[:, :nb * 128],
                            in1=rel_s[:, base + t:base + t + nb]
                                .to_broadcast([128, nb, 128]),
                            op=mybir.AluOpType.is_equal,
                        )
                        for k in range(nb):
                            nc.tensor.matmul(
                                ps[:],
                                lhsT=s_oh[:, k * 128:(k + 1) * 128],
                                rhs=mg[:, (t + k) * width:(t + k + 1) * width],
                                start=(t + k == 0),
                                stop=(t + k == tw - 1),
                            )
                        t += nb
                    epilogue(w, ps, pseps, eppool)

        # ---------------- Layer 1 ----------------
        def epi1(w, ps, pseps, eppool):
            z = eppool.tile([128, HID], dt.float32, tag="z1")
            nc.vector.tensor_add(z[:], ps[:], u_acc[:, w * HID:(w + 1) * HID])
            h = eppool.tile([128, HID], dt.float32, tag="h1")
            if with_b1:
                zz = eppool.tile([128, HID], dt.float32, tag="zz1")
                nc.vector.tensor_scalar_mul(zz[:], z[:], dinv[:, w:w + 1])
                nc.vector.tensor_add(zz[:], zz[:], b1_s[:])
                nc.scalar.activation(h[:], zz[:], mybir.ActivationFunctionType.Tanh)
            else:
                nc.scalar.activation(
                    h[:], z[:], mybir.ActivationFunctionType.Tanh,
                    scale=dinv[:, w:w + 1],
                )
            # table_2 row block: (dinv * h) @ W2, via PE transpose
            th = eppool.tile([128, HID], dt.float32, tag="th")
            nc.vector.tensor_scalar_mul(th[:], h[:], dinv[:, w:w + 1])
            pt = pseps.tile([HID, 128], dt.float32, tag="tr1")
            nc.tensor.transpose(pt[:], th[:], ident[:])
            thT = eppool.tile([HID, 128], dt.float32, tag="thT")
            nc.vector.tensor_copy(thT[:], pt[:])
            p2 = pseps.tile([128, OUT2], dt.float32, tag="t2")
            nc.tensor.matmul(p2[:], lhsT=thT[:], rhs=W2_s[:], start=True, stop=True)
            nc.vector.tensor_copy(t2_acc[:, w * OUT2:(w + 1) * OUT2], p2[:])

        aggregate(u_tab.ap(), HID, epi1)

        # publish + allgather the layer-2 table (HWDGE; no cast needed)
        nc.sync.dma_start(t2_shard.ap(), t2_acc[:])
        if single_core:
            for c in range(NCORES):
                nc.sync.dma_start(
                    t2_tab.ap()[c * NPAD:(c + 1) * NPAD, :], t2_shard.ap()
                )
        else:
            nc.gpsimd.collective_compute(
                "AllGather", mybir.AluOpType.bypass, replica_groups=groups,
                ins=[t2_shard.ap()], outs=[t2_tab.ap()],
            )

        # ---------------- Layer 2 ----------------
        def epi2(w, ps, pseps, eppool):
            z = eppool.tile([128, OUT2], dt.float32, tag="z2")
            nc.vector.tensor_add(z[:], ps[:], t2_acc[:, w * OUT2:(w + 1) * OUT2])
            h2 = h2_acc[:, w * OUT2:(w + 1) * OUT2]
            if with_b2:
                zz = eppool.tile([128, OUT2], dt.float32, tag="zz2")
                nc.vector.tensor_scalar_mul(zz[:], z[:], dinv[:, w:w + 1])
                nc.vector.tensor_add(zz[:], zz[:], b2_s[:])
                nc.scalar.activation(h2, zz[:], mybir.ActivationFunctionType.Tanh)
            else:
                nc.scalar.activation(
                    h2, z[:], mybir.ActivationFunctionType.Tanh,
                    scale=dinv[:, w:w + 1],
                )
            # out = h2 @ Wc + bc, via PE transpose
            pt = pseps.tile([OUT2, 128], dt.float32, tag="tr2")
            nc.tensor.transpose(pt[:], h2, ident[:])
            h2T = eppool.tile([OUT2, 128], dt.float32, tag="h2T")
            nc.vector.tensor_copy(h2T[:], pt[:])
            pf = pseps.tile([NCLS, 128], dt.float32, tag="fin")
            nc.tensor.matmul(pf[:], lhsT=Wc_s[:], rhs=h2T[:], start=True, stop=True)
            nc.scalar.activation(
                out_acc[:, w * 128:(w + 1) * 128], pf[:],
                mybir.ActivationFunctionType.Identity, bias=bc_s[:],
            )

        aggregate(t2_tab.ap(), OUT2, epi2)

        # ---------------- outputs ----------------
        nc.sync.dma_start(outT.ap(), out_acc[:])
        nc.sync.dma_start(h2o.ap(), h2_acc[:])

    nc.compile()
    return nc


def _get_program(tiles_w, with_b1: bool, with_b2: bool):
    key = (tuple(tiles_w), with_b1, with_b2)
    if key not in _prog_cache:
        _prog_cache[key] = _build_program(tiles_w, with_b1, with_b2)
    return _prog_cache[key]


def _window_counts(dst):
    owner = dst // NLOC
    local = dst - owner * NLOC
    gw = owner * WINS + (local >> 7)
    cnts = np.bincount(gw, minlength=NCORES * WINS).reshape(NCORES, WINS)
    return cnts


def _preprocess(x, edge_index, deg_full, tiles_w):
    """Per-core inputs: xT, offs, rel, deg layouts. Pure index/layout work."""
    tiles_w = np.asarray(tiles_w, np.int64)
    ntiles = int(tiles_w.sum())
    tstart = np.zeros(WINS, np.int64)
    np.cumsum(tiles_w[:-1], out=tstart[1:])

    src = np.asarray(edge_index[0], dtype=np.int64)
    dst = np.asarray(edge_index[1], dtype=np.int64)
    order = np.argsort(dst, kind="stable")
    ds = dst[order]
    ss = src[order]
    core_starts = np.searchsorted(ds, np.arange(NCORES + 1) * NLOC)

    per_core = []
    for c in range(NCORES):
        lo, hi = int(core_starts[c]), int(core_starts[c + 1])
        ldst = ds[lo:hi] - c * NLOC
        lsrc = ss[lo:hi]
        w = ldst >> 7
        relv = (ldst & 127).astype(np.float32)
        cnt = np.bincount(w, minlength=WINS)
        if np.any(cnt > tiles_w * 128):
            raise ValueError("window overflow vs tiles_w")
        wstart = np.zeros(WINS, np.int64)
        np.cumsum(cnt[:-1], out=wstart[1:])
        pos = np.arange(hi - lo, dtype=np.int64) - wstart[w]

        # edge j of window w -> tile t=j//128, partition p=j%128, at column
        # tstart[w] + t of the [128, ntiles] SBUF layout
        col = tstart[w] + (pos >> 7)
        row_p = pos & 127

        oc = lsrc // NLOC
        ol = lsrc % NLOC
        trow = (oc * NPAD + (ol & 127) * WINS + (ol >> 7)).astype(np.int32)

        offs_sb = np.zeros((128, ntiles), np.int32)
        rel_sb = np.full((128, ntiles), PAD_REL, np.float32)
        offs_sb[row_p, col] = trow
        rel_sb[row_p, col] = relv

        deg_lay = np.ones((128, WINS), np.float32)
        l = np.arange(NLOC)
        deg_lay[l & 127, l >> 7] = deg_full[c * NLOC:(c + 1) * NLOC]

        xTc = np.zeros((128, NPAD), np.float32)
        xTc[:, :NLOC] = np.asarray(x[c * NLOC:(c + 1) * NLOC], np.float32).T

        per_core.append(dict(xT=xTc, offs=offs_sb, rel=rel_sb, deg=deg_lay))
    return per_core


def kernel(x, edge_index, W1, b1, W2, b2, Wc, bc, **_unused):
    x = np.asarray(x, np.float32)
    W1 = np.asarray(W1, np.float32)
    b1 = np.asarray(b1, np.float32)
    W2 = np.asarray(W2, np.float32)
    b2 = np.asarray(b2, np.float32)
    Wc = np.asarray(Wc, np.float32)
    bc = np.asarray(bc, np.float32)
    dst = np.asarray(edge_index[1], dtype=np.int64)

    deg_full = (np.bincount(dst, minlength=N) + 1).astype(np.float32)

    cnts = _window_counts(dst)
    tiles_w = tuple(
        max(1, int(t)) for t in np.ceil(cnts.max(axis=0) / 128).astype(int)
    )

    with_b1 = bool(np.any(b1))
    with_b2 = bool(np.any(b2))
    nc = _get_program(tiles_w, with_b1, with_b2)

    per_core = _preprocess(x, edge_index, deg_full, tiles_w)

    in_maps = []
    for c in range(NCORES):
        m = dict(
            xT=per_core[c]["xT"],
            offs=per_core[c]["offs"],
            rel=per_core[c]["rel"],
            deg=per_core[c]["deg"],
            W1t=W1,
            W2t=W2,
            Wct=Wc,
            bct=bc.reshape(NCLS, 1),
        )
        if with_b1:
            m["b1t"] = np.broadcast_to(b1, (128, HID)).copy()
        if with_b2:
            m["b2t"] = np.broadcast_to(b2, (128, OUT2)).copy()
        in_maps.append(m)

    res = bass_utils.run_bass_kernel_spmd(nc, in_maps, core_ids=list(range(NCORES)))

    out = np.empty((N, NCLS), np.float32)
    h2 = np.empty((N, OUT2), np.float32)
    for c in range(NCORES):
        r = res.results[c]
        out[c * NLOC:(c + 1) * NLOC] = r["outT"].T[:NLOC]
        h2c = r["h2o"].reshape(128, WINS, OUT2).transpose(1, 0, 2).reshape(NPAD, OUT2)
        h2[c * NLOC:(c + 1) * NLOC] = h2c[:NLOC]
    return (out, h2)


# revision 16
# speedup vs baseline: 1.0169x; 1.0168x over previous
"""Trainium2 Bass kernel for a 2-layer GCN (PyG GCNConv semantics).

Strategy (8 NeuronCores, node-sharded / graph parallel):
  - Core c owns nodes [c*12500, (c+1)*12500), padded to 12544 = 98 windows x 128.
  - Using linearity of GCNConv, both layers become:
        z = dinv * (segment_sum_{e: dst=d} table[src_e] + table[d])  (+bias)
    with  table_1 = dinv * (x @ W1)         (64 wide)
          h       = tanh(z_1)
          table_2 = (dinv * h) @ W2         (2 wide)
          h2      = tanh(z_2)
          out     = h2 @ Wc + bc
    so there is NO per-edge scalar work at all - just gather + segmented sum.
  - Each core computes the table rows for its own nodes, AllGathers the full
    table to DRAM, then processes its in-edges (sorted by dst) in 128-node
    windows: per 128 edges one [128,1]-offset indirect DMA gathers the source
    rows (the only reliable data-dependent DMA on this platform), and a
    0/1-matrix matmul accumulates the segment sums for the window's 128
    nodes in PSUM.
  - The graph is known at program-build time, so every window emits exactly
    ceil(max-over-cores edge count / 128) tiles; padding lanes carry
    slot id 512 so their one-hot column is all-zero (contribute nothing).

Host-side work is restricted to graph partitioning / index packing (sorting
edges by destination, window slot assignment, degree counts) and final
output reassembly.
"""

import sys

sys.path.insert(0, "/opt/trn_rl_repo")

import numpy as np
from contextlib import ExitStack

from concourse import bacc, mybir, tile
from concourse import bass_utils
from concourse.bass import IndirectOffsetOnAxis
from concourse.masks import make_identity

F32 = np.float32

# Problem geometry (hardcoded per contest contract).
N = 100000
F_IN = 128
HID = 64
OUT2 = 2
NCLS = 4
NCORES = 8
NLOC = N // NCORES          # 12500 nodes per core
WINS = (NLOC + 127) // 128  # 98 windows of 128 nodes
NPAD = WINS * 128           # 12544 rows per core in the padded table
SB = 12                     # one-hot build batch (tiles per DVE op)
PAD_REL = 512.0             # slot id for padding edges -> all-zero one-hot col

_prog_cache: dict = {}


def _build_program(tiles_w, with_b1: bool, with_b2: bool, single_core: bool = False):
    """Build + compile the SPMD Bass program.

    tiles_w: per-window 128-edge tile counts (len WINS tuple, shared by all
    cores = max over cores).  single_core=True builds a collective-free
    variant (AllGather modeled as 8 local DRAM copies) for TimelineSim.
    """
    tiles_w = tuple(int(t) for t in tiles_w)
    ntiles = sum(tiles_w)
    tstart = np.concatenate([[0], np.cumsum(tiles_w)]).astype(int)

    dt = mybir.dt
    nc = bacc.Bacc(
        "TRN2",
        target_bir_lowering=False,
        debug=False,
        enable_asserts=False,
        num_devices=1 if single_core else NCORES,
    )

    # ------------- I/O -------------
    xT = nc.dram_tensor("xT", [128, NPAD], dt.float32, kind="ExternalInput")
    offs = nc.dram_tensor("offs", [128, ntiles], dt.int32, kind="ExternalInput")
    rel = nc.dram_tensor("rel", [128, ntiles], dt.float32, kind="ExternalInput")
    relB = nc.dram_tensor("relB", [128, ntiles], dt.float32, kind="ExternalInput")
    deg = nc.dram_tensor("deg", [128, WINS], dt.float32, kind="ExternalInput")
    W1t = nc.dram_tensor("W1t", [128, HID], dt.float32, kind="ExternalInput")
    W2t = nc.dram_tensor("W2t", [HID, OUT2], dt.float32, kind="ExternalInput")
    Wct = nc.dram_tensor("Wct", [OUT2, NCLS], dt.float32, kind="ExternalInput")
    bct = nc.dram_tensor("bct", [NCLS, 1], dt.float32, kind="ExternalInput")
    if with_b1:
        b1t = nc.dram_tensor("b1t", [128, HID], dt.float32, kind="ExternalInput")
    if with_b2:
        b2t = nc.dram_tensor("b2t", [128, OUT2], dt.float32, kind="ExternalInput")

    outT = nc.dram_tensor("outT", [NCLS, NPAD], dt.float32, kind="ExternalOutput")
    h2o = nc.dram_tensor("h2o", [128, WINS * OUT2], dt.float32, kind="ExternalOutput")

    # Internal tables. Shard layout: row (p*WINS + w) holds node w*128+p.
    u_shard = nc.dram_tensor("u_shard", [NPAD, HID], dt.float32, kind="Internal")
    u_tab = nc.dram_tensor(
        "u_tab", [NCORES * NPAD, HID], dt.float32, kind="Internal",
        addr_space="Shared",
    )
    t2_shard = nc.dram_tensor("t2_shard", [NPAD, OUT2], dt.float32, kind="Internal")
    t2_tab = nc.dram_tensor(
        "t2_tab", [NCORES * NPAD, OUT2], dt.float32, kind="Internal",
        addr_space="Shared",
    )

    groups = [list(range(NCORES))]

    with tile.TileContext(nc) as tc, ExitStack() as ctx:
        const = ctx.enter_context(tc.tile_pool(name="const", bufs=1))

        # --- constants / persistent SBUF state ---
        W1_s = const.tile([128, HID], dt.float32)
        nc.sync.dma_start(W1_s[:], W1t.ap())
        W2_s = const.tile([HID, OUT2], dt.float32)
        nc.sync.dma_start(W2_s[:], W2t.ap())
        Wc_s = const.tile([OUT2, NCLS], dt.float32)
        nc.sync.dma_start(Wc_s[:], Wct.ap())
        bc_s = const.tile([NCLS, 1], dt.float32)
        nc.sync.dma_start(bc_s[:], bct.ap())
        if with_b1:
            b1_s = const.tile([128, HID], dt.float32)
            nc.sync.dma_start(b1_s[:], b1t.ap())
        if with_b2:
            b2_s = const.tile([128, OUT2], dt.float32)
            nc.sync.dma_start(b2_s[:], b2t.ap())

        offs_s = const.tile([128, ntiles], dt.int32)
        nc.sync.dma_start(offs_s[:], offs.ap())
        rel_s = const.tile([128, ntiles], dt.float32)
        nc.sync.dma_start(rel_s[:], rel.ap())
        relB_s = const.tile([128, ntiles], dt.float32)
        nc.sync.dma_start(relB_s[:], relB.ap())

        deg_s = const.tile([128, WINS], dt.float32)
        nc.sync.dma_start(deg_s[:], deg.ap())
        sqd = const.tile([128, WINS], dt.float32)
        nc.scalar.activation(sqd[:], deg_s[:], mybir.ActivationFunctionType.Sqrt)
        dinv = const.tile([128, WINS], dt.float32)
        nc.vector.reciprocal(dinv[:], sqd[:])

        # iota pattern repeated SB times: iota_rep[p, b*128 + s] = s
        iota16 = const.tile([128, SB * 128], dt.int16)
        nc.gpsimd.iota(iota16[:], pattern=[[0, SB], [1, 128]], base=0,
                       channel_multiplier=0)
        iota_f = const.tile([128, SB * 128], dt.float32)
        nc.vector.tensor_copy(iota_f[:], iota16[:])

        ident = const.tile([128, 128], dt.float32)
        make_identity(nc, ident[:])

        u_acc = const.tile([128, WINS * HID], dt.float32)     # own table rows (L1)
        t2_acc = const.tile([128, WINS * OUT2], dt.float32)   # own table rows (L2)
        h2_acc = const.tile([128, WINS * OUT2], dt.float32)   # layer-2 tanh output
        out_acc = const.tile([NCLS, NPAD], dt.float32)        # final logits (T)

        # ---------------- Phase A: table_1 = dinv * (x @ W1) ----------------
        with tc.tile_pool(name="phA_ps", bufs=4, space="PSUM") as psA, \
             tc.tile_pool(name="phA_sb", bufs=1) as sbA:
            xT_s = sbA.tile([128, NPAD], dt.float32)
            nc.sync.dma_start(xT_s[:], xT.ap())
            for w in range(WINS):
                ps = psA.tile([128, HID], dt.float32)
                nc.tensor.matmul(
                    ps[:], lhsT=xT_s[:, w * 128:(w + 1) * 128], rhs=W1_s[:],
                    start=True, stop=True,
                )
                nc.vector.tensor_scalar_mul(
                    u_acc[:, w * HID:(w + 1) * HID], ps[:], dinv[:, w:w + 1]
                )
        # publish + allgather the layer-1 table (HWDGE; no cast needed)
        nc.sync.dma_start(u_shard.ap(), u_acc[:])
        if single_core:
            for c in range(NCORES):
                nc.sync.dma_start(
                    u_tab.ap()[c * NPAD:(c + 1) * NPAD, :], u_shard.ap()
                )
        else:
            nc.gpsimd.collective_compute(
                "AllGather", mybir.AluOpType.bypass, replica_groups=groups,
                ins=[u_shard.ap()], outs=[u_tab.ap()],
            )

        # ---------------- Aggregation helper ----------------
        def aggregate(tab_ap, width, epilogue):
            with tc.tile_pool(name="agg_g", bufs=5) as gpool, \
                 tc.tile_pool(name="agg_s", bufs=4) as spool, \
                 tc.tile_pool(name="agg_sb", bufs=2) as sbpool, \
                 tc.tile_pool(name="agg_ps", bufs=4, space="PSUM") as pspool, \
                 tc.tile_pool(name="agg_eps", bufs=2, space="PSUM") as pseps, \
                 tc.tile_pool(name="agg_ep", bufs=3) as eppool:
                for w in range(WINS):
                    tw = tiles_w[w]
                    base = tstart[w]
                    mg = gpool.tile([128, tw * width], dt.float32, tag="mg")
                    for t in range(tw):
                        nc.gpsimd.indirect_dma_start(
                            out=mg[:, t * width:(t + 1) * width],
                            out_offset=None,
                            in_=tab_ap,
                            in_offset=IndirectOffsetOnAxis(
                                ap=offs_s[:, base + t:base + t + 1], axis=0
                            ),
                        )
                    ps = pspool.tile([128, width], dt.float32, tag="acc")
                    t = 0
                    while t < tw:
                        nb = min(SB, tw - t)
                        s_oh = spool.tile([128, SB * 128], dt.float32, tag="oh")
                        s_b = sbpool.tile([128, SB * 128], dt.float32, tag="ohb")
                        nc.vector.tensor_tensor(
                            out=s_oh[:, :nb * 128],
                            in0=iota_f[:, :nb * 128],
                            in1=rel_s[:, base + t:base + t + nb]
                                .to_broadcast([128, nb, 128]),
                            op=mybir.AluOpType.is_equal,
                        )
                        nc.vector.tensor_tensor(
                            out=s_b[:, :nb * 128],
                            in0=iota_f[:, :nb * 128],
                            in1=relB_s[:, base + t:base + t + nb]
                                .to_broadcast([128, nb, 128]),
                            op=mybir.AluOpType.is_equal,
                        )
                        nc.vector.tensor_tensor(
                            out=s_oh[:, :nb * 128],
                            in0=s_oh[:, :nb * 128],
                            in1=s_b[:, :nb * 128],
                            op=mybir.AluOpType.add,
                        )
                        for k in range(nb):
                            nc.tensor.matmul(
                                ps[:],
                                lhsT=s_oh[:, k * 128:(k + 1) * 128],
                                rhs=mg[:, (t + k) * width:(t + k + 1) * width],
                                start=(t + k == 0),
                                stop=(t + k == tw - 1),
                            )
                        t += nb
                    epilogue(w, ps, pseps, eppool)

        # ---------------- Layer 1 ----------------
        def epi1(w, ps, pseps, eppool):
            z = eppool.tile([128, HID], dt.float32, tag="z1")
            nc.vector.tensor_add(z[:], ps[:], u_acc[:, w * HID:(w + 1) * HID])
            h = eppool.tile([128, HID], dt.float32, tag="h1")
            if with_b1:
                zz = eppool.tile([128, HID], dt.float32, tag="zz1")
                nc.vector.tensor_scalar_mul(zz[:], z[:], dinv[:, w:w + 1])
                nc.vector.tensor_add(zz[:], zz[:], b1_s[:])
                nc.scalar.activation(h[:], zz[:], mybir.ActivationFunctionType.Tanh)
            else:
                nc.scalar.activation(
                    h[:], z[:], mybir.ActivationFunctionType.Tanh,
                    scale=dinv[:, w:w + 1],
                )
            # table_2 row block: (dinv * h) @ W2, via PE transpose
            th = eppool.tile([128, HID], dt.float32, tag="th")
            nc.vector.tensor_scalar_mul(th[:], h[:], dinv[:, w:w + 1])
            pt = pseps.tile([HID, 128], dt.float32, tag="tr1")
            nc.tensor.transpose(pt[:], th[:], ident[:])
            thT = eppool.tile([HID, 128], dt.float32, tag="thT")
            nc.vector.tensor_copy(thT[:], pt[:])
            p2 = pseps.tile([128, OUT2], dt.float32, tag="t2")
            nc.tensor.matmul(p2[:], lhsT=thT[:], rhs=W2_s[:], start=True, stop=True)
            nc.vector.tensor_copy(t2_acc[:, w * OUT2:(w + 1) * OUT2], p2[:])

        aggregate(u_tab.ap(), HID, epi1)

        # publish + allgather the layer-2 table (HWDGE; no cast needed)
        nc.sync.dma_start(t2_shard.ap(), t2_acc[:])
        if single_core:
            for c in range(NCORES):
                nc.sync.dma_start(
                    t2_tab.ap()[c * NPAD:(c + 1) * NPAD, :], t2_shard.ap()
                )
        else:
            nc.gpsimd.collective_compute(
                "AllGather", mybir.AluOpType.bypass, replica_groups=groups,
                ins=[t2_shard.ap()], outs=[t2_tab.ap()],
            )

        # ---------------- Layer 2 ----------------
        def epi2(w, ps, pseps, eppool):
            z = eppool.tile([128, OUT2], dt.float32, tag="z2")
            nc.vector.tensor_add(z[:], ps[:], t2_acc[:, w * OUT2:(w + 1) * OUT2])
            h2 = h2_acc[:, w * OUT2:(w + 1) * OUT2]
            if with_b2:
                zz = eppool.tile([128, OUT2], dt.float32, tag="zz2")
                nc.vector.tensor_scalar_mul(zz[:], z[:], dinv[:, w:w + 1])
                nc.vector.tensor_add(zz[:], zz[:], b2_s[:])
                nc.scalar.activation(h2, zz[:], mybir.ActivationFunctionType.Tanh)
            else:
                nc.scalar.activation(
                    h2, z[:], mybir.ActivationFunctionType.Tanh,
                    scale=dinv[:, w:w + 1],
                )
            # out = h2 @ Wc + bc, via PE transpose
            pt = pseps.tile([OUT2, 128], dt.float32, tag="tr2")
            nc.tensor.transpose(pt[:], h2, ident[:])
            h2T = eppool.tile([OUT2, 128], dt.float32, tag="h2T")
            nc.vector.tensor_copy(h2T[:], pt[:])
            pf = pseps.tile([NCLS, 128], dt.float32, tag="fin")
            nc.tensor.matmul(pf[:], lhsT=Wc_s[:], rhs=h2T[:], start=True, stop=True)
            nc.scalar.activation(
                out_acc[:, w * 128:(w + 1) * 128], pf[:],
                mybir.ActivationFunctionType.Identity, bias=bc_s[:],
            )

        aggregate(t2_tab.ap(), OUT2, epi2)

        # ---------------- outputs ----------------
        nc.sync.dma_start(outT.ap(), out_acc[:])
        nc.sync.dma_start(h2o.ap(), h2_acc[:])

    nc.compile()
    return nc


def _get_program(tiles_w, with_b1: bool, with_b2: bool):
    key = (tuple(tiles_w), with_b1, with_b2)
    if key not in _prog_cache:
        _prog_cache[key] = _build_program(tiles_w, with_b1, with_b2)
    return _prog_cache[key]


def _window_counts(dst, src):
    """Paired gather-position counts per (core, window): sources repeated
    within a window share a gathered row (two slot ids per row)."""
    owner = dst // NLOC
    local = dst - owner * NLOC
    gw = owner * WINS + (local >> 7)
    key = (gw.astype(np.int64) << 17) | src
    uq, cnt = np.unique(key, return_counts=True)
    pos = (cnt + 1) >> 1
    gw_u = (uq >> 17).astype(np.int64)
    pcnt = np.bincount(gw_u, weights=pos.astype(np.float64),
                       minlength=NCORES * WINS).astype(np.int64)
    return pcnt.reshape(NCORES, WINS)


def _preprocess(x, edge_index, deg_full, tiles_w):
    """Per-core inputs: xT, offs, rel, deg layouts. Pure index/layout work."""
    tiles_w = np.asarray(tiles_w, np.int64)
    ntiles = int(tiles_w.sum())
    tstart = np.zeros(WINS, np.int64)
    np.cumsum(tiles_w[:-1], out=tstart[1:])

    src = np.asarray(edge_index[0], dtype=np.int64)
    dst = np.asarray(edge_index[1], dtype=np.int64)
    order = np.argsort(dst, kind="stable")
    ds = dst[order]
    ss = src[order]
    core_starts = np.searchsorted(ds, np.arange(NCORES + 1) * NLOC)

    per_core = []
    for c in range(NCORES):
        lo, hi = int(core_starts[c]), int(core_starts[c + 1])
        n = hi - lo
        ldst = ds[lo:hi] - c * NLOC
        lsrc = ss[lo:hi]
        w = ldst >> 7
        relv = (ldst & 127).astype(np.float32)

        # pair edges sharing (window, src): one gathered row, two slot ids
        key = (w.astype(np.int64) << 17) | lsrc
        order2 = np.argsort(key, kind="stable")
        ks = key[order2]
        new_grp = np.empty(n, bool)
        new_grp[0] = True
        np.not_equal(ks[1:], ks[:-1], out=new_grp[1:])
        grp_first = np.flatnonzero(new_grp)
        grp_id = np.cumsum(new_grp) - 1
        idx_in_grp = np.arange(n, dtype=np.int64) - grp_first[grp_id]
        half = (idx_in_grp & 1).astype(bool)   # False=A, True=B
        is_pos = ~half
        pos_seq = np.cumsum(is_pos) - 1        # B edges inherit partner's id
        w_s = (ks >> 17).astype(np.int64)
        pos_per_w = np.bincount(w_s[is_pos], minlength=WINS)
        if np.any(pos_per_w > tiles_w * 128):
            raise ValueError("window overflow vs tiles_w")
        pw_start = np.zeros(WINS, np.int64)
        np.cumsum(pos_per_w[:-1], out=pw_start[1:])
        pos_in_w = pos_seq - pw_start[w_s]

        col = tstart[w_s] + (pos_in_w >> 7)
        row_p = pos_in_w & 127

        lsrc_s = lsrc[order2]
        rel_srt = relv[order2]
        oc = lsrc_s // NLOC
        ol = lsrc_s % NLOC
        trow = (oc * NPAD + (ol & 127) * WINS + (ol >> 7)).astype(np.int32)

        offs_sb = np.zeros((128, ntiles), np.int32)
        rel_sb = np.full((128, ntiles), PAD_REL, np.float32)
        relB_sb = np.full((128, ntiles), PAD_REL, np.float32)
        a = is_pos
        offs_sb[row_p[a], col[a]] = trow[a]
        rel_sb[row_p[a], col[a]] = rel_srt[a]
        relB_sb[row_p[half], col[half]] = rel_srt[half]

        deg_lay = np.ones((128, WINS), np.float32)
        l = np.arange(NLOC)
        deg_lay[l & 127, l >> 7] = deg_full[c * NLOC:(c + 1) * NLOC]

        xTc = np.zeros((128, NPAD), np.float32)
        xTc[:, :NLOC] = np.asarray(x[c * NLOC:(c + 1) * NLOC], np.float32).T

        per_core.append(dict(xT=xTc, offs=offs_sb, rel=rel_sb,
                             relB=relB_sb, deg=deg_lay))
    return per_core


def kernel(x, edge_index, W1, b1, W2, b2, Wc, bc, **_unused):
    x = np.asarray(x, np.float32)
    W1 = np.asarray(W1, np.float32)
    b1 = np.asarray(b1, np.float32)
    W2 = np.asarray(W2, np.float32)
    b2 = np.asarray(b2, np.float32)
    Wc = np.asarray(Wc, np.float32)
    bc = np.asarray(bc, np.float32)
    dst = np.asarray(edge_index[1], dtype=np.int64)

    deg_full = (np.bincount(dst, minlength=N) + 1).astype(np.float32)

    src_arr = np.asarray(edge_index[0], dtype=np.int64)
    cnts = _window_counts(dst, src_arr)
    tiles_w = tuple(
        max(1, int(t)) for t in np.ceil(cnts.max(axis=0) / 128).astype(int)
    )

    with_b1 = bool(np.any(b1))
    with_b2 = bool(np.any(b2))
    nc = _get_program(tiles_w, with_b1, with_b2)

    per_core = _preprocess(x, edge_index, deg_full, tiles_w)

    in_maps = []
    for c in range(NCORES):
        m = dict(
            xT=per_core[c]["xT"],
            offs=per_core[c]["offs"],
            rel=per_core[c]["rel"],
            relB=per_core[c]["relB"],
            deg=per_core[c]["deg"],
            W1t=W1,
            W2t=W2,
            Wct=Wc,
            bct=bc.reshape(NCLS, 1),
        )
        if with_b1:
            m["b1t"] = np.broadcast_to(b1, (128, HID)).copy()
        if with_b2:
            m["b2t"] = np.broadcast_to(b2, (128, OUT2)).copy()
        in_maps.append(m)

    res = bass_utils.run_bass_kernel_spmd(nc, in_maps, core_ids=list(range(NCORES)))

    out = np.empty((N, NCLS), np.float32)
    h2 = np.empty((N, OUT2), np.float32)
    for c in range(NCORES):
        r = res.results[c]
        out[c * NLOC:(c + 1) * NLOC] = r["outT"].T[:NLOC]
        h2c = r["h2o"].reshape(128, WINS, OUT2).transpose(1, 0, 2).reshape(NPAD, OUT2)
        h2[c * NLOC:(c + 1) * NLOC] = h2c[:NLOC]
    return (out, h2)


# revision 18
# speedup vs baseline: 1.0506x; 1.0332x over previous
"""Trainium2 Bass kernel for a 2-layer GCN (PyG GCNConv semantics).

Strategy (8 NeuronCores, node-sharded / graph parallel):
  - Core c owns nodes [c*12500, (c+1)*12500), padded to 12544 = 98 windows x 128.
  - Using linearity of GCNConv, both layers become:
        z = dinv * (segment_sum_{e: dst=d} table[src_e] + table[d])  (+bias)
    with  table_1 = dinv * (x @ W1)         (64 wide)
          h       = tanh(z_1)
          table_2 = (dinv * h) @ W2         (2 wide)
          h2      = tanh(z_2)
          out     = h2 @ Wc + bc
    so there is NO per-edge scalar work at all - just gather + segmented sum.
  - Each core computes the table rows for its own nodes, AllGathers the full
    table to DRAM, then processes its in-edges (sorted by dst) in 128-node
    windows: per 128 edges one [128,1]-offset indirect DMA gathers the source
    rows (the only reliable data-dependent DMA on this platform), and a
    0/1-matrix matmul accumulates the segment sums for the window's 128
    nodes in PSUM.
  - The graph is known at program-build time, so every window emits exactly
    ceil(max-over-cores edge count / 128) tiles; padding lanes carry
    slot id 512 so their one-hot column is all-zero (contribute nothing).

Host-side work is restricted to graph partitioning / index packing (sorting
edges by destination, window slot assignment, degree counts) and final
output reassembly.
"""

import sys

sys.path.insert(0, "/opt/trn_rl_repo")

import numpy as np
from contextlib import ExitStack

from concourse import bacc, mybir, tile
from concourse import bass_utils
from concourse.bass import IndirectOffsetOnAxis
from concourse.masks import make_identity

F32 = np.float32

# Problem geometry (hardcoded per contest contract).
N = 100000
F_IN = 128
HID = 64
OUT2 = 2
NCLS = 4
NCORES = 8
NLOC = N // NCORES          # 12500 nodes per core
WINS = (NLOC + 127) // 128  # 98 windows of 128 nodes
NPAD = WINS * 128           # 12544 rows per core in the padded table
SB = 12                     # one-hot build batch (tiles per DVE op)
PAD_REL = 512.0             # slot id for padding edges -> all-zero one-hot col

_prog_cache: dict = {}


def _build_program(tiles_w, with_b1: bool, with_b2: bool, single_core: bool = False):
    """Build + compile the SPMD Bass program.

    tiles_w: per-window 128-edge tile counts (len WINS tuple, shared by all
    cores = max over cores).  single_core=True builds a collective-free
    variant (AllGather modeled as 8 local DRAM copies) for TimelineSim.
    """
    tiles_w = tuple(int(t) for t in tiles_w)
    ntiles = sum(tiles_w)
    tstart = np.concatenate([[0], np.cumsum(tiles_w)]).astype(int)

    dt = mybir.dt
    nc = bacc.Bacc(
        "TRN2",
        target_bir_lowering=False,
        debug=False,
        enable_asserts=False,
        num_devices=1 if single_core else NCORES,
    )

    # ------------- I/O -------------
    xT = nc.dram_tensor("xT", [128, NPAD], dt.float32, kind="ExternalInput")
    offs = nc.dram_tensor("offs", [128, ntiles], dt.int32, kind="ExternalInput")
    rel = nc.dram_tensor("rel", [128, ntiles], dt.float32, kind="ExternalInput")
    relB = nc.dram_tensor("relB", [128, ntiles], dt.float32, kind="ExternalInput")
    deg = nc.dram_tensor("deg", [128, WINS], dt.float32, kind="ExternalInput")
    W1t = nc.dram_tensor("W1t", [128, HID], dt.float32, kind="ExternalInput")
    W2t = nc.dram_tensor("W2t", [HID, OUT2], dt.float32, kind="ExternalInput")
    Wct = nc.dram_tensor("Wct", [OUT2, NCLS], dt.float32, kind="ExternalInput")
    bct = nc.dram_tensor("bct", [NCLS, 1], dt.float32, kind="ExternalInput")
    if with_b1:
        b1t = nc.dram_tensor("b1t", [128, HID], dt.float32, kind="ExternalInput")
    if with_b2:
        b2t = nc.dram_tensor("b2t", [128, OUT2], dt.float32, kind="ExternalInput")

    outT = nc.dram_tensor("outT", [NCLS, NPAD], dt.float32, kind="ExternalOutput")
    h2o = nc.dram_tensor("h2o", [128, WINS * OUT2], dt.float32, kind="ExternalOutput")

    # Internal tables. Shard layout: row (p*WINS + w) holds node w*128+p.
    u_shard = nc.dram_tensor("u_shard", [NPAD, HID], dt.float32, kind="Internal")
    u_tab = nc.dram_tensor(
        "u_tab", [NCORES * NPAD, HID], dt.float32, kind="Internal",
        addr_space="Shared",
    )
    t2_shard = nc.dram_tensor("t2_shard", [NPAD, OUT2], dt.float32, kind="Internal")
    t2_tab = nc.dram_tensor(
        "t2_tab", [NCORES * NPAD, OUT2], dt.float32, kind="Internal",
        addr_space="Shared",
    )

    groups = [list(range(NCORES))]

    with tile.TileContext(nc) as tc, ExitStack() as ctx:
        const = ctx.enter_context(tc.tile_pool(name="const", bufs=1))

        # --- constants / persistent SBUF state ---
        W1_s = const.tile([128, HID], dt.float32)
        nc.sync.dma_start(W1_s[:], W1t.ap())
        W2_s = const.tile([HID, OUT2], dt.float32)
        nc.sync.dma_start(W2_s[:], W2t.ap())
        Wc_s = const.tile([OUT2, NCLS], dt.float32)
        nc.sync.dma_start(Wc_s[:], Wct.ap())
        bc_s = const.tile([NCLS, 1], dt.float32)
        nc.sync.dma_start(bc_s[:], bct.ap())
        if with_b1:
            b1_s = const.tile([128, HID], dt.float32)
            nc.sync.dma_start(b1_s[:], b1t.ap())
        if with_b2:
            b2_s = const.tile([128, OUT2], dt.float32)
            nc.sync.dma_start(b2_s[:], b2t.ap())

        offs_s = const.tile([128, ntiles], dt.int32)
        nc.sync.dma_start(offs_s[:], offs.ap())
        rel_s = const.tile([128, ntiles], dt.float32)
        nc.sync.dma_start(rel_s[:], rel.ap())
        relB_s = const.tile([128, ntiles], dt.float32)
        nc.sync.dma_start(relB_s[:], relB.ap())

        deg_s = const.tile([128, WINS], dt.float32)
        nc.sync.dma_start(deg_s[:], deg.ap())
        sqd = const.tile([128, WINS], dt.float32)
        nc.scalar.activation(sqd[:], deg_s[:], mybir.ActivationFunctionType.Sqrt)
        dinv = const.tile([128, WINS], dt.float32)
        nc.vector.reciprocal(dinv[:], sqd[:])

        # iota pattern repeated SB2 times: iota_rep[p, b*256 + s] = s (s<256)
        SB2 = SB // 2
        iota16 = const.tile([128, SB2 * 256], dt.int16)
        nc.gpsimd.iota(iota16[:], pattern=[[0, SB2], [1, 256]], base=0,
                       channel_multiplier=0)
        iota_f = const.tile([128, SB2 * 256], dt.float32)
        nc.vector.tensor_copy(iota_f[:], iota16[:])

        ident = const.tile([128, 128], dt.float32)
        make_identity(nc, ident[:])

        u_acc = const.tile([128, WINS * HID], dt.float32)     # own table rows (L1)
        t2_acc = const.tile([128, WINS * OUT2], dt.float32)   # own table rows (L2)
        h2_acc = const.tile([128, WINS * OUT2], dt.float32)   # layer-2 tanh output
        out_acc = const.tile([NCLS, NPAD], dt.float32)        # final logits (T)

        # ---------------- Phase A: table_1 = dinv * (x @ W1) ----------------
        with tc.tile_pool(name="phA_ps", bufs=4, space="PSUM") as psA, \
             tc.tile_pool(name="phA_sb", bufs=1) as sbA:
            xT_s = sbA.tile([128, NPAD], dt.float32)
            nc.sync.dma_start(xT_s[:], xT.ap())
            for w in range(WINS):
                ps = psA.tile([128, HID], dt.float32)
                nc.tensor.matmul(
                    ps[:], lhsT=xT_s[:, w * 128:(w + 1) * 128], rhs=W1_s[:],
                    start=True, stop=True,
                )
                nc.vector.tensor_scalar_mul(
                    u_acc[:, w * HID:(w + 1) * HID], ps[:], dinv[:, w:w + 1]
                )
        # publish + allgather the layer-1 table (HWDGE; no cast needed)
        nc.sync.dma_start(u_shard.ap(), u_acc[:])
        if single_core:
            for c in range(NCORES):
                nc.sync.dma_start(
                    u_tab.ap()[c * NPAD:(c + 1) * NPAD, :], u_shard.ap()
                )
        else:
            nc.gpsimd.collective_compute(
                "AllGather", mybir.AluOpType.bypass, replica_groups=groups,
                ins=[u_shard.ap()], outs=[u_tab.ap()],
            )

        # ---------------- Aggregation helper ----------------
        def aggregate(tab_ap, width, epilogue):
            with tc.tile_pool(name="agg_g", bufs=3) as gpool, \
                 tc.tile_pool(name="agg_s", bufs=2) as spool, \
                 tc.tile_pool(name="agg_sb", bufs=2) as sbpool, \
                 tc.tile_pool(name="agg_ps", bufs=4, space="PSUM") as pspool, \
                 tc.tile_pool(name="agg_eps", bufs=2, space="PSUM") as pseps, \
                 tc.tile_pool(name="agg_ep", bufs=3) as eppool:
                for sw in range(WINS // 2):
                    tw = tiles_w[sw]
                    base = tstart[sw]
                    mg = gpool.tile([128, tw * width], dt.float32, tag="mg")
                    for t in range(tw):
                        nc.gpsimd.indirect_dma_start(
                            out=mg[:, t * width:(t + 1) * width],
                            out_offset=None,
                            in_=tab_ap,
                            in_offset=IndirectOffsetOnAxis(
                                ap=offs_s[:, base + t:base + t + 1], axis=0
                            ),
                        )
                    ps0 = pspool.tile([128, width], dt.float32, tag="acc")
                    ps1 = pspool.tile([128, width], dt.float32, tag="acc")
                    t = 0
                    while t < tw:
                        nb = min(SB2, tw - t)
                        s_oh = spool.tile([128, SB2 * 256], dt.float32, tag="oh")
                        s_b = sbpool.tile([128, SB2 * 256], dt.float32, tag="ohb")
                        nc.vector.tensor_tensor(
                            out=s_oh[:, :nb * 256],
                            in0=iota_f[:, :nb * 256],
                            in1=rel_s[:, base + t:base + t + nb]
                                .to_broadcast([128, nb, 256]),
                            op=mybir.AluOpType.is_equal,
                        )
                        nc.vector.tensor_tensor(
                            out=s_b[:, :nb * 256],
                            in0=iota_f[:, :nb * 256],
                            in1=relB_s[:, base + t:base + t + nb]
                                .to_broadcast([128, nb, 256]),
                            op=mybir.AluOpType.is_equal,
                        )
                        nc.vector.tensor_tensor(
                            out=s_oh[:, :nb * 256],
                            in0=s_oh[:, :nb * 256],
                            in1=s_b[:, :nb * 256],
                            op=mybir.AluOpType.add,
                        )
                        for k in range(nb):
                            nc.tensor.matmul(
                                ps0[:],
                                lhsT=s_oh[:, k * 256:k * 256 + 128],
                                rhs=mg[:, (t + k) * width:(t + k + 1) * width],
                                start=(t + k == 0),
                                stop=(t + k == tw - 1),
                            )
                            nc.tensor.matmul(
                                ps1[:],
                                lhsT=s_oh[:, k * 256 + 128:(k + 1) * 256],
                                rhs=mg[:, (t + k) * width:(t + k + 1) * width],
                                start=(t + k == 0),
                                stop=(t + k == tw - 1),
                            )
                        t += nb
                    epilogue(2 * sw, ps0, pseps, eppool)
                    epilogue(2 * sw + 1, ps1, pseps, eppool)

        # ---------------- Layer 1 ----------------
        def epi1(w, ps, pseps, eppool):
            z = eppool.tile([128, HID], dt.float32, tag="z1")
            nc.vector.tensor_add(z[:], ps[:], u_acc[:, w * HID:(w + 1) * HID])
            h = eppool.tile([128, HID], dt.float32, tag="h1")
            if with_b1:
                zz = eppool.tile([128, HID], dt.float32, tag="zz1")
                nc.vector.tensor_scalar_mul(zz[:], z[:], dinv[:, w:w + 1])
                nc.vector.tensor_add(zz[:], zz[:], b1_s[:])
                nc.scalar.activation(h[:], zz[:], mybir.ActivationFunctionType.Tanh)
            else:
                nc.scalar.activation(
                    h[:], z[:], mybir.ActivationFunctionType.Tanh,
                    scale=dinv[:, w:w + 1],
                )
            # table_2 row block: (dinv * h) @ W2, via PE transpose
            th = eppool.tile([128, HID], dt.float32, tag="th")
            nc.vector.tensor_scalar_mul(th[:], h[:], dinv[:, w:w + 1])
            pt = pseps.tile([HID, 128], dt.float32, tag="tr1")
            nc.tensor.transpose(pt[:], th[:], ident[:])
            thT = eppool.tile([HID, 128], dt.float32, tag="thT")
            nc.vector.tensor_copy(thT[:], pt[:])
            p2 = pseps.tile([128, OUT2], dt.float32, tag="t2")
            nc.tensor.matmul(p2[:], lhsT=thT[:], rhs=W2_s[:], start=True, stop=True)
            nc.vector.tensor_copy(t2_acc[:, w * OUT2:(w + 1) * OUT2], p2[:])

        aggregate(u_tab.ap(), HID, epi1)

        # publish + allgather the layer-2 table (HWDGE; no cast needed)
        nc.sync.dma_start(t2_shard.ap(), t2_acc[:])
        if single_core:
            for c in range(NCORES):
                nc.sync.dma_start(
                    t2_tab.ap()[c * NPAD:(c + 1) * NPAD, :], t2_shard.ap()
                )
        else:
            nc.gpsimd.collective_compute(
                "AllGather", mybir.AluOpType.bypass, replica_groups=groups,
                ins=[t2_shard.ap()], outs=[t2_tab.ap()],
            )

        # ---------------- Layer 2 ----------------
        def epi2(w, ps, pseps, eppool):
            z = eppool.tile([128, OUT2], dt.float32, tag="z2")
            nc.vector.tensor_add(z[:], ps[:], t2_acc[:, w * OUT2:(w + 1) * OUT2])
            h2 = h2_acc[:, w * OUT2:(w + 1) * OUT2]
            if with_b2:
                zz = eppool.tile([128, OUT2], dt.float32, tag="zz2")
                nc.vector.tensor_scalar_mul(zz[:], z[:], dinv[:, w:w + 1])
                nc.vector.tensor_add(zz[:], zz[:], b2_s[:])
                nc.scalar.activation(h2, zz[:], mybir.ActivationFunctionType.Tanh)
            else:
                nc.scalar.activation(
                    h2, z[:], mybir.ActivationFunctionType.Tanh,
                    scale=dinv[:, w:w + 1],
                )
            # out = h2 @ Wc + bc, via PE transpose
            pt = pseps.tile([OUT2, 128], dt.float32, tag="tr2")
            nc.tensor.transpose(pt[:], h2, ident[:])
            h2T = eppool.tile([OUT2, 128], dt.float32, tag="h2T")
            nc.vector.tensor_copy(h2T[:], pt[:])
            pf = pseps.tile([NCLS, 128], dt.float32, tag="fin")
            nc.tensor.matmul(pf[:], lhsT=Wc_s[:], rhs=h2T[:], start=True, stop=True)
            nc.scalar.activation(
                out_acc[:, w * 128:(w + 1) * 128], pf[:],
                mybir.ActivationFunctionType.Identity, bias=bc_s[:],
            )

        aggregate(t2_tab.ap(), OUT2, epi2)

        # ---------------- outputs ----------------
        nc.sync.dma_start(outT.ap(), out_acc[:])
        nc.sync.dma_start(h2o.ap(), h2_acc[:])

    nc.compile()
    return nc


def _get_program(tiles_w, with_b1: bool, with_b2: bool):
    key = (tuple(tiles_w), with_b1, with_b2)
    if key not in _prog_cache:
        _prog_cache[key] = _build_program(tiles_w, with_b1, with_b2)
    return _prog_cache[key]


def _window_counts(dst, src):
    """Paired gather-position counts per (core, window): sources repeated
    within a window share a gathered row (two slot ids per row)."""
    owner = dst // NLOC
    local = dst - owner * NLOC
    gw = owner * (WINS // 2) + (local >> 8)
    key = (gw.astype(np.int64) << 17) | src
    uq, cnt = np.unique(key, return_counts=True)
    pos = (cnt + 1) >> 1
    gw_u = (uq >> 17).astype(np.int64)
    pcnt = np.bincount(gw_u, weights=pos.astype(np.float64),
                       minlength=NCORES * (WINS // 2)).astype(np.int64)
    return pcnt.reshape(NCORES, WINS // 2)


def _preprocess(x, edge_index, deg_full, tiles_w):
    """Per-core inputs: xT, offs, rel, deg layouts. Pure index/layout work."""
    tiles_w = np.asarray(tiles_w, np.int64)
    ntiles = int(tiles_w.sum())
    tstart = np.zeros(WINS // 2, np.int64)
    np.cumsum(tiles_w[:-1], out=tstart[1:])

    src = np.asarray(edge_index[0], dtype=np.int64)
    dst = np.asarray(edge_index[1], dtype=np.int64)
    order = np.argsort(dst, kind="stable")
    ds = dst[order]
    ss = src[order]
    core_starts = np.searchsorted(ds, np.arange(NCORES + 1) * NLOC)

    per_core = []
    for c in range(NCORES):
        lo, hi = int(core_starts[c]), int(core_starts[c + 1])
        n = hi - lo
        ldst = ds[lo:hi] - c * NLOC
        lsrc = ss[lo:hi]
        w = ldst >> 8
        relv = (ldst & 255).astype(np.float32)

        # pair edges sharing (super-window, src): one gathered row, 2 slot ids
        key = (w.astype(np.int64) << 17) | lsrc
        order2 = np.argsort(key, kind="stable")
        ks = key[order2]
        new_grp = np.empty(n, bool)
        new_grp[0] = True
        np.not_equal(ks[1:], ks[:-1], out=new_grp[1:])
        grp_first = np.flatnonzero(new_grp)
        grp_id = np.cumsum(new_grp) - 1
        idx_in_grp = np.arange(n, dtype=np.int64) - grp_first[grp_id]
        half = (idx_in_grp & 1).astype(bool)   # False=A, True=B
        is_pos = ~half
        pos_seq = np.cumsum(is_pos) - 1        # B edges inherit partner's id
        w_s = (ks >> 17).astype(np.int64)
        pos_per_w = np.bincount(w_s[is_pos], minlength=WINS // 2)
        if np.any(pos_per_w > tiles_w * 128):
            raise ValueError("window overflow vs tiles_w")
        pw_start = np.zeros(WINS // 2, np.int64)
        np.cumsum(pos_per_w[:-1], out=pw_start[1:])
        pos_in_w = pos_seq - pw_start[w_s]

        col = tstart[w_s] + (pos_in_w >> 7)
        row_p = pos_in_w & 127

        lsrc_s = lsrc[order2]
        rel_srt = relv[order2]
        oc = lsrc_s // NLOC
        ol = lsrc_s % NLOC
        trow = (oc * NPAD + (ol & 127) * WINS + (ol >> 7)).astype(np.int32)

        offs_sb = np.zeros((128, ntiles), np.int32)
        rel_sb = np.full((128, ntiles), PAD_REL, np.float32)
        relB_sb = np.full((128, ntiles), PAD_REL, np.float32)
        a = is_pos
        offs_sb[row_p[a], col[a]] = trow[a]
        rel_sb[row_p[a], col[a]] = rel_srt[a]
        relB_sb[row_p[half], col[half]] = rel_srt[half]

        deg_lay = np.ones((128, WINS), np.float32)
        l = np.arange(NLOC)
        deg_lay[l & 127, l >> 7] = deg_full[c * NLOC:(c + 1) * NLOC]

        xTc = np.zeros((128, NPAD), np.float32)
        xTc[:, :NLOC] = np.asarray(x[c * NLOC:(c + 1) * NLOC], np.float32).T

        per_core.append(dict(xT=xTc, offs=offs_sb, rel=rel_sb,
                             relB=relB_sb, deg=deg_lay))
    return per_core


def kernel(x, edge_index, W1, b1, W2, b2, Wc, bc, **_unused):
    x = np.asarray(x, np.float32)
    W1 = np.asarray(W1, np.float32)
    b1 = np.asarray(b1, np.float32)
    W2 = np.asarray(W2, np.float32)
    b2 = np.asarray(b2, np.float32)
    Wc = np.asarray(Wc, np.float32)
    bc = np.asarray(bc, np.float32)
    dst = np.asarray(edge_index[1], dtype=np.int64)

    deg_full = (np.bincount(dst, minlength=N) + 1).astype(np.float32)

    src_arr = np.asarray(edge_index[0], dtype=np.int64)
    cnts = _window_counts(dst, src_arr)
    tiles_w = tuple(
        max(1, int(t)) for t in np.ceil(cnts.max(axis=0) / 128).astype(int)
    )

    with_b1 = bool(np.any(b1))
    with_b2 = bool(np.any(b2))
    nc = _get_program(tiles_w, with_b1, with_b2)

    per_core = _preprocess(x, edge_index, deg_full, tiles_w)

    in_maps = []
    for c in range(NCORES):
        m = dict(
            xT=per_core[c]["xT"],
            offs=per_core[c]["offs"],
            rel=per_core[c]["rel"],
            relB=per_core[c]["relB"],
            deg=per_core[c]["deg"],
            W1t=W1,
            W2t=W2,
            Wct=Wc,
            bct=bc.reshape(NCLS, 1),
        )
        if with_b1:
            m["b1t"] = np.broadcast_to(b1, (128, HID)).copy()
        if with_b2:
            m["b2t"] = np.broadcast_to(b2, (128, OUT2)).copy()
        in_maps.append(m)

    res = bass_utils.run_bass_kernel_spmd(nc, in_maps, core_ids=list(range(NCORES)))

    out = np.empty((N, NCLS), np.float32)
    h2 = np.empty((N, OUT2), np.float32)
    for c in range(NCORES):
        r = res.results[c]
        out[c * NLOC:(c + 1) * NLOC] = r["outT"].T[:NLOC]
        h2c = r["h2o"].reshape(128, WINS, OUT2).transpose(1, 0, 2).reshape(NPAD, OUT2)
        h2[c * NLOC:(c + 1) * NLOC] = h2c[:NLOC]
    return (out, h2)
